# revision 26
# baseline (speedup 1.0000x reference)
"""Trainium2 Bass kernel for nn_EnhancedSpatialAttention.

Full (unsharded) inputs in, full output out. Internally: pure data-parallel
across 8 NeuronCores (2 batch samples per core), one Bass program run SPMD.

Per-sample layout on a core: x_s = [C=128 partitions, H=64, W=512].

x is cast fp32->bf16 during the input DMA (SWDGE cast); all matmuls run in
bf16 (fp32 matmuls on trn2 run in LOW_HIGH mode at ~3.4 cycles/column).
PSUM accumulation stays fp32, the softmax chain and the sigmoid/apply
stay fp32.

Per sample:
  pools:  h_max/w_max via DVE tensor_reduce; h_sum/w_sum via PE
          identity-matmul accumulation into PSUM (mean 1/N folded into the
          conv weights host-side)
  convs:  block-diagonal [128x128] bf16 stationaries (4 parts share
          weights), 6 accumulating matmuls each for ph / pw
  softmax(relu(z)): exp computed as sigmoid(z)/sigmoid(-z) so the whole
          kernel stays in the single "sigmoid" ACT table set
  amap:   per-h stationary (w_e^T * phn[:,h]) matmul against pwn
  apply:  out = x * (1 + sigmoid(amap + b_e)); sigmoid+1 on ACT,
          multiply on GPSIMD tensor_tensor (bf16 x, fp32 t, fp32 out)
"""

import os
import sys
import types
import contextlib

for _p in ("/opt/trn_rl_repo", "/root/.axon_site/_ro/trn_rl_repo"):
    if os.path.isdir(_p) and _p not in sys.path:
        sys.path.insert(0, _p)

import numpy as np
import ml_dtypes

import concourse.bass as bass
import concourse.tile as tile
from concourse import mybir
from concourse.tile import ScopedClock
import concourse.bass_utils as bass_utils
from concourse.bass_utils import run_bass_kernel_spmd

AF = mybir.ActivationFunctionType
ALU = mybir.AluOpType
FP32 = mybir.dt.float32
BF16 = mybir.dt.bfloat16
BF16NP = ml_dtypes.bfloat16

NCORES = 8
B, C, H, W = 16, 128, 64, 512
PC = 32          # channels per part
NPARTS = 4
BPC = B // NCORES  # samples per core
HW = H * W
HALF = HW // 2   # elements per half-sample (h rows 0..31 / 32..63)
HH = H // 2


def _patch_drain_split():
    """This container's walrus accepts only one sync-wait command per
    instruction; Tile's end-of-kernel drain carries one wait per live
    semaphore. Spread them across SP nops, one wait each."""
    if getattr(tile.TileContext, "_drain_split_patched", False):
        return

    def _drain_and_barrier_split(self, tick_clock, wait_clock):
        nc = self.nc
        probe = nc.sync.nop(hint="drain_wait_probe", nofuse=True)
        wait_clock.add_sem_waits(
            probe.ins, ScopedClock({None: tick_clock.global_clock})
        )
        waits = list(probe.ins.sync_info.on_wait or [])
        probe.ins.sync_info.on_wait = waits[:1]
        for w in waits[1:]:
            n = nc.sync.nop(hint="drain_wait_split", nofuse=True)
            n.ins.sync_info = mybir.SyncInfo(on_wait=[w], on_update=[])
        nc.sync.drain()
        nc.all_engine_barrier()
        assert self.sems is not None
        popped = nc._tile_sem_poison_stack.pop()
        assert popped is self._sem_poison
        nc.clear_and_free_semaphores(list(self.sems.allocated().values()))
        nc.all_engine_barrier()

    tile.TileContext._drain_and_barrier = _drain_and_barrier_split
    tile.TileContext._drain_split_patched = True


def _split_sync_waits(nc, max_waits=1):
    """This walrus build accepts at most one sync-wait command per
    instruction. Hoist extra waits onto same-engine NoOps inserted just
    before the instruction (the engine stalls on each in turn, which is
    semantically identical)."""
    for fn in nc.m.functions:
        for blk in fn.blocks:
            new = []
            for inst in blk.instructions:
                si = inst.sync_info
                if (si is not None and si.on_wait
                        and len(si.on_wait) > max_waits
                        and inst.engine != mybir.EngineType.Unassigned):
                    waits = list(si.on_wait)
                    for w in waits[max_waits:]:
                        nop = mybir.InstNoOp(
                            name=nc.get_next_instruction_name(),
                            engine=inst.engine,
                            ins=[], outs=[],
                            sync_info=mybir.SyncInfo(on_wait=[w], on_update=[]),
                        )
                        nc.register_instruction(nop, overwrite=True)
                        new.append(nop)
                    si.on_wait = waits[:max_waits]
                new.append(inst)
            blk.instructions[:] = new


def _dedupe_ldweights(nc):
    """Consecutive matmuls that reuse the same stationary (the identity for
    the pool-sum accumulations) do not need to reload it: LDWEIGHTS of the
    same rows serializes against the in-flight matmul, so each redundant
    reload costs a full array drain. Drop repeated LDWEIGHTS whose weights
    AP is identical to the previous one on the PE stream, carrying any
    sync waits onto the next kept PE instruction."""
    for fn in nc.m.functions:
        for blk in fn.blocks:
            new = []
            last_key = None
            pending = []
            for inst in blk.instructions:
                if inst.engine == mybir.EngineType.PE:
                    tn = type(inst).__name__
                    if tn == "InstLdweights":
                        a = inst.ins[0]
                        key = (getattr(a, "memref", None), a.offset,
                               str(a.ap), str(a.dtype))
                        has_upd = bool(inst.sync_info
                                       and inst.sync_info.on_update)
                        if key == last_key and not has_upd:
                            if inst.sync_info and inst.sync_info.on_wait:
                                pending.extend(inst.sync_info.on_wait)
                            continue
                        last_key = key
                    elif tn != "InstMatmult":
                        last_key = None
                    if pending:
                        si = inst.sync_info
                        if si is None:
                            inst.sync_info = mybir.SyncInfo(
                                on_wait=list(pending), on_update=[])
                        else:
                            si.on_wait = list(si.on_wait) + pending
                        pending = []
                new.append(inst)
            blk.instructions[:] = new


def _install_ntff_hook():
    """run_bass_kernel_spmd(trace=True) imports antenv.axon_hooks, which is
    absent in this container; provide it, backed by the ctypes NTFF hook
    from trn_agent_boot. Harmless if tracing is never requested."""
    if "antenv.axon_hooks" in sys.modules:
        return
    mod = types.ModuleType("antenv.axon_hooks")
    holder = [None]
    mod.set_axon_ntff_profile_hook = lambda h: holder.__setitem__(0, h)
    mod.get_axon_ntff_profile_hook = lambda: holder[0]
    sys.modules["antenv.axon_hooks"] = mod
    try:
        from trn_agent_boot.trn_boot import _ntff_profile_via_ctypes

        so = "/opt/axon/libaxon_pjrt.so"
        if os.path.exists(so):
            holder[0] = _ntff_profile_via_ctypes(so)
    except Exception:
        pass
    # upload_artifacts needs S3; keep artifacts local.
    bass_utils.upload_artifacts = lambda tmpdir: "file://" + tmpdir


def _blockdiag4(m32):
    out = np.zeros((128, 128), np.float32)
    for p in range(NPARTS):
        out[p * PC:(p + 1) * PC, p * PC:(p + 1) * PC] = m32
    return out


def _build_program():
    _patch_drain_split()
    nc = bass.Bass()
    xin = nc.declare_dram_parameter("x", [BPC, C, H, W], FP32, isOutput=False)
    bdh_d = nc.declare_dram_parameter("bdh", [C, 6 * 128], BF16, isOutput=False)
    bdw_d = nc.declare_dram_parameter("bdw", [C, 6 * 128], BF16, isOutput=False)
    bde_d = nc.declare_dram_parameter("bde", [C, 128], BF16, isOutput=False)
    iden_d = nc.declare_dram_parameter("iden", [C, 128], BF16, isOutput=False)
    bias_d = nc.declare_dram_parameter("bias", [C, 3], FP32, isOutput=False)
    yout = nc.declare_dram_parameter("out", [BPC, C, H, W], FP32, isOutput=True)

    with tile.TileContext(nc) as tc, contextlib.ExitStack() as ctx:
        consts = ctx.enter_context(tc.tile_pool(name="consts", bufs=1))
        xpool = ctx.enter_context(tc.tile_pool(name="xh", bufs=3))
        t1p = ctx.enter_context(tc.tile_pool(name="t1", bufs=2))
        foldp = ctx.enter_context(tc.tile_pool(name="fold", bufs=1))
        outp = ctx.enter_context(tc.tile_pool(name="ost", bufs=3))
        statp = ctx.enter_context(tc.tile_pool(name="stat", bufs=4))
        smallp = ctx.enter_context(tc.tile_pool(name="small", bufs=1))
        ps_wsum_p = ctx.enter_context(
            tc.tile_pool(name="ps_wsum", bufs=1, space="PSUM"))
        ps_mix_p = ctx.enter_context(
            tc.tile_pool(name="ps_mix", bufs=1, space="PSUM"))
        ps_conv_p = ctx.enter_context(
            tc.tile_pool(name="ps_conv", bufs=1, space="PSUM"))
        ps_amap_p = ctx.enter_context(
            tc.tile_pool(name="ps_amap", bufs=2, space="PSUM"))

        # constants
        bdh = consts.tile([128, 6 * 128], BF16)
        nc.gpsimd.dma_start(out=bdh[:], in_=bdh_d[:])
        bdw = consts.tile([128, 6 * 128], BF16)
        nc.gpsimd.dma_start(out=bdw[:], in_=bdw_d[:])
        bde = consts.tile([128, 128], BF16)
        nc.gpsimd.dma_start(out=bde[:], in_=bde_d[:])
        iden = consts.tile([128, 128], BF16)
        nc.gpsimd.dma_start(out=iden[:], in_=iden_d[:])
        biases = consts.tile([128, 3], FP32)
        nc.gpsimd.dma_start(out=biases[:], in_=bias_d[:])
        bh4 = biases[:, 0:1]
        bw4 = biases[:, 1:2]
        be4 = biases[:, 2:3]

        xin2 = [xin[s].rearrange("c h w -> c (h w)") for s in range(BPC)]
        yout2 = [yout[s].rearrange("c h w -> c (h w)") for s in range(BPC)]

        # All input DMAs issued upfront (SWDGE drains its queue FIFO, so
        # sample 0's halves complete first; SDMA transfer is async).
        # fp32 -> bf16 cast happens inline in the DMA.
        xh = {}
        QRT = HALF // 8
        for s in range(BPC):
            for hf in range(2):
                xt = xpool.tile([128, HALF], BF16, tag="xh")
                for q in range(8):
                    nc.gpsimd.dma_start(
                        out=xt[:, q * QRT:(q + 1) * QRT],
                        in_=xin2[s][:, hf * HALF + q * QRT:
                                    hf * HALF + (q + 1) * QRT])
                xh[(s, hf)] = xt

        for s in range(BPC):
            # ---------------- reduce phase ----------------
            # pool buffers (zero-padded boundary columns)
            ph_sum = smallp.tile([128, H + 2], BF16, tag="ph_sum")
            ph_max = smallp.tile([128, H + 2], BF16, tag="ph_max")
            pw_sum = smallp.tile([128, W + 2], BF16, tag="pw_sum")
            pw_max = smallp.tile([128, W + 2], BF16, tag="pw_max")
            for t, n in ((ph_sum, H), (ph_max, H), (pw_sum, W), (pw_max, W)):
                nc.vector.memset(t[:, 0:1], 0.0)
                nc.vector.memset(t[:, n + 1:n + 2], 0.0)

            ps_wsum = ps_wsum_p.tile([128, W], FP32, tag="ps_wsum")
            wmax_part = []
            for hf in range(2):
                xt = xh[(s, hf)]
                x3 = xt.rearrange("p (h w) -> p h w", h=HH)
                # h_max rows for this half
                nc.vector.tensor_reduce(
                    out=ph_max[:, 1 + hf * HH:1 + (hf + 1) * HH],
                    in_=x3, axis=mybir.AxisListType.X, op=ALU.max)
                # w_max partial via pairwise fold tree over h (contiguous)
                fa = foldp.tile([128, HALF // 2], BF16, tag="fold_a")
                nc.vector.tensor_tensor(
                    out=fa[:], in0=xt[:, :HALF // 2], in1=xt[:, HALF // 2:],
                    op=ALU.max)
                fb = foldp.tile([128, HALF // 4], BF16, tag="fold_b")
                nc.vector.tensor_tensor(
                    out=fb[:], in0=fa[:, :HALF // 4], in1=fa[:, HALF // 4:],
                    op=ALU.max)
                fc = foldp.tile([128, HALF // 8], BF16, tag="fold_c")
                nc.vector.tensor_tensor(
                    out=fc[:], in0=fb[:, :HALF // 8], in1=fb[:, HALF // 8:],
                    op=ALU.max)
                fd = foldp.tile([128, HALF // 16], BF16, tag="fold_d")
                nc.vector.tensor_tensor(
                    out=fd[:], in0=fc[:, :HALF // 16], in1=fc[:, HALF // 16:],
                    op=ALU.max)
                pm = smallp.tile([128, W], BF16, tag=f"wmax{hf}")
                nc.vector.tensor_tensor(
                    out=pm[:], in0=fd[:, :W], in1=fd[:, W:], op=ALU.max)
                wmax_part.append(pm)
                # w_sum: accumulate h-row slices
                for h in range(HH):
                    nc.tensor.matmul(
                        ps_wsum[:], iden[:], x3[:, h, :],
                        start=(hf == 0 and h == 0),
                        stop=(hf == 1 and h == HH - 1))
                # h_sum: 32 accumulating matmuls of N=512,
                # psum col = i2*256 + g*32 + h
                ps_h8 = ps_mix_p.tile([128, 2, 8, HH], FP32, tag="mix")
                xv = xt.rearrange("p (h g ii i2) -> p ii i2 g h",
                                  g=8, ii=32, i2=2)
                for ii in range(32):
                    nc.tensor.matmul(
                        ps_h8[:], iden[:], xv[:, ii],
                        start=(ii == 0), stop=(ii == 31))
                # combine the 16 (i2, g) partials -> h_sum rows
                h8v = ps_h8.rearrange("p i2 g h -> p h (i2 g)")
                with nc.allow_low_precision("bf16 pools by design"):
                    nc.vector.tensor_reduce(
                        out=ph_sum[:, 1 + hf * HH:1 + (hf + 1) * HH],
                        in_=h8v, axis=mybir.AxisListType.X, op=ALU.add)
            # w_max combine + w_sum pair-combine
            nc.vector.tensor_tensor(
                out=pw_max[:, 1:1 + W], in0=wmax_part[0][:],
                in1=wmax_part[1][:], op=ALU.max)
            nc.scalar.activation(
                out=pw_sum[:, 1:1 + W], in_=ps_wsum[:], func=AF.Copy)

            # ---------------- convs + softmax ----------------
            ps_ph = ps_mix_p.tile([128, H], FP32, tag="mix")
            ps_pw = ps_conv_p.tile([128, W], FP32, tag="ps_pw")
            for j, (dh, k) in enumerate(
                    (dh, k) for dh in range(3) for k in range(2)):
                pool = ph_sum if k == 0 else ph_max
                nc.tensor.matmul(
                    ps_ph[:], bdh[:, j * 128:(j + 1) * 128],
                    pool[:, dh:dh + H], start=(j == 0), stop=(j == 5))
            for kh in range(2):
                for dw in range(3):
                    pool = pw_sum if kh == 0 else pw_max
                    jj = kh * 3 + dw
                    nc.tensor.matmul(
                        ps_pw[:], bdw[:, jj * 128:(jj + 1) * 128],
                        pool[:, dw:dw + W], start=(jj == 0), stop=(jj == 5))

            def softmax_relu(ps, bias_ap, n, tagp, out_dtype):
                z = smallp.tile([128, n], FP32, tag=f"z{tagp}")
                nc.scalar.activation(out=z[:], in_=ps[:], func=AF.Relu,
                                     bias=bias_ap, scale=1.0)
                sp = smallp.tile([128, n], FP32, tag=f"sp{tagp}")
                nc.scalar.activation(out=sp[:], in_=z[:], func=AF.Sigmoid)
                sn = smallp.tile([128, n], FP32, tag=f"sn{tagp}")
                nc.scalar.activation(out=sn[:], in_=z[:], func=AF.Sigmoid,
                                     scale=-1.0)
                nc.vector.reciprocal(out=sn[:], in_=sn[:])
                e = smallp.tile([128, n], FP32, tag=f"e{tagp}")
                nc.vector.tensor_tensor(out=e[:], in0=sp[:], in1=sn[:],
                                        op=ALU.mult)
                ssum = smallp.tile([128, 1], FP32, tag=f"ss{tagp}")
                nc.vector.tensor_reduce(out=ssum[:], in_=e[:],
                                        axis=mybir.AxisListType.X, op=ALU.add)
                nc.vector.reciprocal(out=ssum[:], in_=ssum[:])
                pn = smallp.tile([128, n], out_dtype, tag=f"pn{tagp}")
                nc.scalar.activation(out=pn[:], in_=e[:], func=AF.Copy,
                                     scale=ssum[:])
                return pn

            phn = softmax_relu(ps_ph, bh4, H, "h", FP32)
            pwn = softmax_relu(ps_pw, bw4, W, "w", BF16)

            # ---------------- amap + apply ----------------
            for k in range(32):  # 2 h-rows per psum chunk
                am = ps_amap_p.tile([128, 1024], FP32, tag="am")
                for hl in range(2):
                    h = 2 * k + hl
                    stat = statp.tile([128, 128], BF16, tag="stat")
                    nc.scalar.activation(out=stat[:], in_=bde[:], func=AF.Copy,
                                         scale=phn[:, h:h + 1])
                    nc.tensor.matmul(
                        am[:, hl * 512:(hl + 1) * 512], stat[:], pwn[:],
                        start=True, stop=True)
                nc.scalar.activation(out=am[:], in_=am[:], func=AF.Sigmoid,
                                     bias=be4, scale=1.0)
                if k % 2 == 0:
                    ost = outp.tile([128, 2048], FP32, tag="ost")
                hf, off = k // 16, (2 * k % 32) * 512
                nc.vector.scalar_tensor_tensor(
                    out=ost[:, (k % 2) * 1024:(k % 2 + 1) * 1024],
                    in0=am[:], scalar=1.0,
                    in1=xh[(s, hf)][:, off:off + 1024],
                    op0=ALU.add, op1=ALU.mult)
                if k % 2 == 1:
                    q = k // 2
                    nc.sync.dma_start(
                        out=yout2[s][:, q * 2048:(q + 1) * 2048], in_=ost[:])
    _dedupe_ldweights(nc)
    _split_sync_waits(nc)
    return nc


_CACHED = {}


def _get_program():
    if "nc" not in _CACHED:
        _install_ntff_hook()
        _CACHED["nc"] = _build_program()
    return _CACHED["nc"]


def _host_weights(w_h, b_h, w_w, b_w, w_e, b_e):
    # mean-pool 1/N scaling folded into the "avg" conv weight blocks
    bdh = np.empty((C, 6 * 128), np.float32)
    for dh in range(3):
        for k in range(2):
            m = w_h[:, :, dh, k].T.astype(np.float32)  # [c_in, o]
            if k == 0:
                m = m / np.float32(W)
            bdh[:, (dh * 2 + k) * 128:(dh * 2 + k + 1) * 128] = _blockdiag4(m)
    bdw = np.empty((C, 6 * 128), np.float32)
    for kh in range(2):
        for dw in range(3):
            m = w_w[:, :, kh, dw].T.astype(np.float32)
            if kh == 0:
                m = m / np.float32(H)
            jj = kh * 3 + dw
            bdw[:, jj * 128:(jj + 1) * 128] = _blockdiag4(m)
    bde = _blockdiag4(w_e[:, :, 0, 0].T.astype(np.float32))
    iden = np.eye(128, dtype=np.float32)
    bias = np.stack([np.tile(b_h, NPARTS), np.tile(b_w, NPARTS),
                     np.tile(b_e, NPARTS)], axis=1).astype(np.float32)
    return dict(bdh=bdh.astype(BF16NP), bdw=bdw.astype(BF16NP),
                bde=bde.astype(BF16NP), iden=iden.astype(BF16NP), bias=bias)


def _run(x, w_h, b_h, w_w, b_w, w_e, b_e, trace=False, tmpdir=None):
    nc = _get_program()
    wts = _host_weights(w_h, b_h, w_w, b_w, w_e, b_e)
    x = np.ascontiguousarray(x, dtype=np.float32)
    in_maps = []
    for i in range(NCORES):
        m = {"x": x[i * BPC:(i + 1) * BPC]}
        m.update(wts)
        in_maps.append(m)
    res = run_bass_kernel_spmd(nc, in_maps, list(range(NCORES)),
                               trace=trace, tmpdir=tmpdir)
    out = np.concatenate([res.results[i]["out"] for i in range(NCORES)],
                         axis=0)
    return out, res


def kernel(x, w_h, b_h, w_w, b_w, w_e, b_e):
    out, _ = _run(x, w_h, b_h, w_w, b_w, w_e, b_e, trace=False)
    return out


# revision 28
# speedup vs baseline: 1.0032x; 1.0032x over previous
"""Trainium2 Bass kernel for nn_EnhancedSpatialAttention.

Full (unsharded) inputs in, full output out. Internally: pure data-parallel
across 8 NeuronCores (2 batch samples per core), one Bass program run SPMD.

Per-sample layout on a core: x_s = [C=128 partitions, H=64, W=512].

x is cast fp32->bf16 during the input DMA (SWDGE cast); all matmuls run in
bf16 (fp32 matmuls on trn2 run in LOW_HIGH mode at ~3.4 cycles/column).
PSUM accumulation stays fp32, the softmax chain and the sigmoid/apply
stay fp32.

Per sample:
  pools:  h_max/w_max via DVE tensor_reduce; h_sum/w_sum via PE
          identity-matmul accumulation into PSUM (mean 1/N folded into the
          conv weights host-side)
  convs:  block-diagonal [128x128] bf16 stationaries (4 parts share
          weights), 6 accumulating matmuls each for ph / pw
  softmax(relu(z)): exp computed as sigmoid(z)/sigmoid(-z) so the whole
          kernel stays in the single "sigmoid" ACT table set
  amap:   per-h stationary (w_e^T * phn[:,h]) matmul against pwn
  apply:  out = x * (1 + sigmoid(amap + b_e)); sigmoid+1 on ACT,
          multiply on GPSIMD tensor_tensor (bf16 x, fp32 t, fp32 out)
"""

import os
import sys
import types
import contextlib

for _p in ("/opt/trn_rl_repo", "/root/.axon_site/_ro/trn_rl_repo"):
    if os.path.isdir(_p) and _p not in sys.path:
        sys.path.insert(0, _p)

import numpy as np
import ml_dtypes

import concourse.bass as bass
import concourse.tile as tile
from concourse import mybir
from concourse.tile import ScopedClock
import concourse.bass_utils as bass_utils
from concourse.bass_utils import run_bass_kernel_spmd

AF = mybir.ActivationFunctionType
ALU = mybir.AluOpType
FP32 = mybir.dt.float32
BF16 = mybir.dt.bfloat16
BF16NP = ml_dtypes.bfloat16

NCORES = 8
B, C, H, W = 16, 128, 64, 512
PC = 32          # channels per part
NPARTS = 4
BPC = B // NCORES  # samples per core
HW = H * W
HALF = HW // 2   # elements per half-sample (h rows 0..31 / 32..63)
HH = H // 2


def _patch_drain_split():
    """This container's walrus accepts only one sync-wait command per
    instruction; Tile's end-of-kernel drain carries one wait per live
    semaphore. Spread them across SP nops, one wait each."""
    if getattr(tile.TileContext, "_drain_split_patched", False):
        return

    def _drain_and_barrier_split(self, tick_clock, wait_clock):
        nc = self.nc
        probe = nc.sync.nop(hint="drain_wait_probe", nofuse=True)
        wait_clock.add_sem_waits(
            probe.ins, ScopedClock({None: tick_clock.global_clock})
        )
        waits = list(probe.ins.sync_info.on_wait or [])
        probe.ins.sync_info.on_wait = waits[:1]
        for w in waits[1:]:
            n = nc.sync.nop(hint="drain_wait_split", nofuse=True)
            n.ins.sync_info = mybir.SyncInfo(on_wait=[w], on_update=[])
        nc.sync.drain()
        nc.all_engine_barrier()
        assert self.sems is not None
        popped = nc._tile_sem_poison_stack.pop()
        assert popped is self._sem_poison
        nc.clear_and_free_semaphores(list(self.sems.allocated().values()))
        nc.all_engine_barrier()

    tile.TileContext._drain_and_barrier = _drain_and_barrier_split
    tile.TileContext._drain_split_patched = True


def _split_sync_waits(nc, max_waits=1):
    """This walrus build accepts at most one sync-wait command per
    instruction. Hoist extra waits onto same-engine NoOps inserted just
    before the instruction (the engine stalls on each in turn, which is
    semantically identical)."""
    for fn in nc.m.functions:
        for blk in fn.blocks:
            new = []
            for inst in blk.instructions:
                si = inst.sync_info
                if (si is not None and si.on_wait
                        and len(si.on_wait) > max_waits
                        and inst.engine != mybir.EngineType.Unassigned):
                    waits = list(si.on_wait)
                    for w in waits[max_waits:]:
                        nop = mybir.InstNoOp(
                            name=nc.get_next_instruction_name(),
                            engine=inst.engine,
                            ins=[], outs=[],
                            sync_info=mybir.SyncInfo(on_wait=[w], on_update=[]),
                        )
                        nc.register_instruction(nop, overwrite=True)
                        new.append(nop)
                    si.on_wait = waits[:max_waits]
                new.append(inst)
            blk.instructions[:] = new


def _dedupe_ldweights(nc):
    """Consecutive matmuls that reuse the same stationary (the identity for
    the pool-sum accumulations) do not need to reload it: LDWEIGHTS of the
    same rows serializes against the in-flight matmul, so each redundant
    reload costs a full array drain. Drop repeated LDWEIGHTS whose weights
    AP is identical to the previous one on the PE stream, carrying any
    sync waits onto the next kept PE instruction."""
    for fn in nc.m.functions:
        for blk in fn.blocks:
            new = []
            last_key = None
            pending = []
            for inst in blk.instructions:
                if inst.engine == mybir.EngineType.PE:
                    tn = type(inst).__name__
                    if tn == "InstLdweights":
                        a = inst.ins[0]
                        key = (getattr(a, "memref", None), a.offset,
                               str(a.ap), str(a.dtype))
                        has_upd = bool(inst.sync_info
                                       and inst.sync_info.on_update)
                        if key == last_key and not has_upd:
                            if inst.sync_info and inst.sync_info.on_wait:
                                pending.extend(inst.sync_info.on_wait)
                            continue
                        last_key = key
                    elif tn != "InstMatmult":
                        last_key = None
                    if pending:
                        si = inst.sync_info
                        if si is None:
                            inst.sync_info = mybir.SyncInfo(
                                on_wait=list(pending), on_update=[])
                        else:
                            si.on_wait = list(si.on_wait) + pending
                        pending = []
                new.append(inst)
            blk.instructions[:] = new


def _install_ntff_hook():
    """run_bass_kernel_spmd(trace=True) imports antenv.axon_hooks, which is
    absent in this container; provide it, backed by the ctypes NTFF hook
    from trn_agent_boot. Harmless if tracing is never requested."""
    if "antenv.axon_hooks" in sys.modules:
        return
    mod = types.ModuleType("antenv.axon_hooks")
    holder = [None]
    mod.set_axon_ntff_profile_hook = lambda h: holder.__setitem__(0, h)
    mod.get_axon_ntff_profile_hook = lambda: holder[0]
    sys.modules["antenv.axon_hooks"] = mod
    try:
        from trn_agent_boot.trn_boot import _ntff_profile_via_ctypes

        so = "/opt/axon/libaxon_pjrt.so"
        if os.path.exists(so):
            holder[0] = _ntff_profile_via_ctypes(so)
    except Exception:
        pass
    # upload_artifacts needs S3; keep artifacts local.
    bass_utils.upload_artifacts = lambda tmpdir: "file://" + tmpdir


def _blockdiag4(m32):
    out = np.zeros((128, 128), np.float32)
    for p in range(NPARTS):
        out[p * PC:(p + 1) * PC, p * PC:(p + 1) * PC] = m32
    return out


def _build_program():
    _patch_drain_split()
    nc = bass.Bass()
    xin = nc.declare_dram_parameter("x", [BPC, C, H, W], FP32, isOutput=False)
    bdh_d = nc.declare_dram_parameter("bdh", [C, 6 * 128], BF16, isOutput=False)
    bdw_d = nc.declare_dram_parameter("bdw", [C, 6 * 128], BF16, isOutput=False)
    bde_d = nc.declare_dram_parameter("bde", [C, 128], BF16, isOutput=False)
    iden_d = nc.declare_dram_parameter("iden", [C, 128], BF16, isOutput=False)
    bias_d = nc.declare_dram_parameter("bias", [C, 3], FP32, isOutput=False)
    yout = nc.declare_dram_parameter("out", [BPC, C, H, W], FP32, isOutput=True)

    with tile.TileContext(nc) as tc, contextlib.ExitStack() as ctx:
        consts = ctx.enter_context(tc.tile_pool(name="consts", bufs=1))
        xpool = ctx.enter_context(tc.tile_pool(name="xh", bufs=3))
        t1p = ctx.enter_context(tc.tile_pool(name="t1", bufs=2))
        foldp = ctx.enter_context(tc.tile_pool(name="fold", bufs=1))
        outp = ctx.enter_context(tc.tile_pool(name="ost", bufs=3))
        statp = ctx.enter_context(tc.tile_pool(name="stat", bufs=4))
        smallp = ctx.enter_context(tc.tile_pool(name="small", bufs=1))
        ps_wsum_p = ctx.enter_context(
            tc.tile_pool(name="ps_wsum", bufs=1, space="PSUM"))
        ps_conv_p = ctx.enter_context(
            tc.tile_pool(name="ps_conv", bufs=1, space="PSUM"))
        ps_amap_p = ctx.enter_context(
            tc.tile_pool(name="ps_amap", bufs=2, space="PSUM"))

        # constants
        bdh = consts.tile([128, 6 * 128], BF16)
        nc.gpsimd.dma_start(out=bdh[:], in_=bdh_d[:])
        bdw = consts.tile([128, 6 * 128], BF16)
        nc.gpsimd.dma_start(out=bdw[:], in_=bdw_d[:])
        bde = consts.tile([128, 128], BF16)
        nc.gpsimd.dma_start(out=bde[:], in_=bde_d[:])
        iden = consts.tile([128, 128], BF16)
        nc.gpsimd.dma_start(out=iden[:], in_=iden_d[:])
        biases = consts.tile([128, 3], FP32)
        nc.gpsimd.dma_start(out=biases[:], in_=bias_d[:])
        bh4 = biases[:, 0:1]
        bw4 = biases[:, 1:2]
        be4 = biases[:, 2:3]

        xin2 = [xin[s].rearrange("c h w -> c (h w)") for s in range(BPC)]
        yout2 = [yout[s].rearrange("c h w -> c (h w)") for s in range(BPC)]

        # All input DMAs issued upfront (SWDGE drains its queue FIFO, so
        # sample 0's halves complete first; SDMA transfer is async).
        # fp32 -> bf16 cast happens inline in the DMA.
        xh = {}
        QRT = HALF // 8
        for s in range(BPC):
            for hf in range(2):
                xt = xpool.tile([128, HALF], BF16, tag="xh")
                for q in range(8):
                    nc.gpsimd.dma_start(
                        out=xt[:, q * QRT:(q + 1) * QRT],
                        in_=xin2[s][:, hf * HALF + q * QRT:
                                    hf * HALF + (q + 1) * QRT])
                xh[(s, hf)] = xt

        for s in range(BPC):
            # ---------------- reduce phase ----------------
            # pool buffers (zero-padded boundary columns)
            ph_sum = smallp.tile([128, H + 2], BF16, tag="ph_sum")
            ph_max = smallp.tile([128, H + 2], BF16, tag="ph_max")
            pw_sum = smallp.tile([128, W + 2], BF16, tag="pw_sum")
            pw_max = smallp.tile([128, W + 2], BF16, tag="pw_max")
            for t, n in ((ph_sum, H), (ph_max, H), (pw_sum, W), (pw_max, W)):
                nc.vector.memset(t[:, 0:1], 0.0)
                nc.vector.memset(t[:, n + 1:n + 2], 0.0)

            ps_wsum = ps_wsum_p.tile([128, W], FP32, tag="ps_wsum")
            wmax_part = []
            for hf in range(2):
                xt = xh[(s, hf)]
                x3 = xt.rearrange("p (h w) -> p h w", h=HH)
                # h_max rows for this half: fold w 512->64, then reduce
                ha = foldp.tile([128, HALF // 2], BF16, tag="fold_a")
                h3a = ha.rearrange("p (h w) -> p h w", h=HH)
                nc.vector.tensor_tensor(
                    out=h3a[:], in0=x3[:, :, 0:256], in1=x3[:, :, 256:512],
                    op=ALU.max)
                hb = foldp.tile([128, HALF // 4], BF16, tag="fold_b")
                h3b = hb.rearrange("p (h w) -> p h w", h=HH)
                nc.vector.tensor_tensor(
                    out=h3b[:], in0=h3a[:, :, 0:128], in1=h3a[:, :, 128:256],
                    op=ALU.max)
                hc = foldp.tile([128, HALF // 8], BF16, tag="fold_c")
                h3c = hc.rearrange("p (h w) -> p h w", h=HH)
                nc.vector.tensor_tensor(
                    out=h3c[:], in0=h3b[:, :, 0:64], in1=h3b[:, :, 64:128],
                    op=ALU.max)
                nc.vector.tensor_reduce(
                    out=ph_max[:, 1 + hf * HH:1 + (hf + 1) * HH],
                    in_=h3c, axis=mybir.AxisListType.X, op=ALU.max)
                # w_max partial via pairwise fold tree over h (contiguous)
                fa = foldp.tile([128, HALF // 2], BF16, tag="fold_a")
                nc.vector.tensor_tensor(
                    out=fa[:], in0=xt[:, :HALF // 2], in1=xt[:, HALF // 2:],
                    op=ALU.max)
                fb = foldp.tile([128, HALF // 4], BF16, tag="fold_b")
                nc.vector.tensor_tensor(
                    out=fb[:], in0=fa[:, :HALF // 4], in1=fa[:, HALF // 4:],
                    op=ALU.max)
                fc = foldp.tile([128, HALF // 8], BF16, tag="fold_c")
                nc.vector.tensor_tensor(
                    out=fc[:], in0=fb[:, :HALF // 8], in1=fb[:, HALF // 8:],
                    op=ALU.max)
                fd = foldp.tile([128, HALF // 16], BF16, tag="fold_d")
                nc.vector.tensor_tensor(
                    out=fd[:], in0=fc[:, :HALF // 16], in1=fc[:, HALF // 16:],
                    op=ALU.max)
                pm = smallp.tile([128, W], BF16, tag=f"wmax{hf}")
                nc.vector.tensor_tensor(
                    out=pm[:], in0=fd[:, :W], in1=fd[:, W:], op=ALU.max)
                wmax_part.append(pm)
                # w_sum: accumulate h-row slices
                for h in range(HH):
                    nc.tensor.matmul(
                        ps_wsum[:], iden[:], x3[:, h, :],
                        start=(hf == 0 and h == 0),
                        stop=(hf == 1 and h == HH - 1))
                # h_sum rows on DVE: tensor_scalar (bf16 4x mode) with
                # fp32 accum_out per row
                hs = smallp.tile([128, HH], FP32, tag=f"hs{hf}")
                trash = smallp.tile([128, W], BF16, tag="trash")
                for h in range(HH):
                    nc.vector.tensor_scalar(
                        out=trash[:], in0=x3[:, h, :], scalar1=1.0,
                        scalar2=0.0, op0=ALU.mult, op1=ALU.add,
                        accum_out=hs[:, h:h + 1])
                nc.scalar.activation(
                    out=ph_sum[:, 1 + hf * HH:1 + (hf + 1) * HH],
                    in_=hs[:], func=AF.Copy)
            # w_max combine + w_sum pair-combine
            nc.vector.tensor_tensor(
                out=pw_max[:, 1:1 + W], in0=wmax_part[0][:],
                in1=wmax_part[1][:], op=ALU.max)
            nc.scalar.activation(
                out=pw_sum[:, 1:1 + W], in_=ps_wsum[:], func=AF.Copy)

            # ---------------- convs + softmax ----------------
            ps_ph = ps_conv_p.tile([128, H], FP32, tag="ps_ph")
            ps_pw = ps_conv_p.tile([128, W], FP32, tag="ps_pw")
            for j, (dh, k) in enumerate(
                    (dh, k) for dh in range(3) for k in range(2)):
                pool = ph_sum if k == 0 else ph_max
                nc.tensor.matmul(
                    ps_ph[:], bdh[:, j * 128:(j + 1) * 128],
                    pool[:, dh:dh + H], start=(j == 0), stop=(j == 5))
            for kh in range(2):
                for dw in range(3):
                    pool = pw_sum if kh == 0 else pw_max
                    jj = kh * 3 + dw
                    nc.tensor.matmul(
                        ps_pw[:], bdw[:, jj * 128:(jj + 1) * 128],
                        pool[:, dw:dw + W], start=(jj == 0), stop=(jj == 5))

            def softmax_relu(ps, bias_ap, n, tagp, out_dtype):
                z = smallp.tile([128, n], FP32, tag=f"z{tagp}")
                nc.scalar.activation(out=z[:], in_=ps[:], func=AF.Relu,
                                     bias=bias_ap, scale=1.0)
                sp = smallp.tile([128, n], FP32, tag=f"sp{tagp}")
                nc.scalar.activation(out=sp[:], in_=z[:], func=AF.Sigmoid)
                sn = smallp.tile([128, n], FP32, tag=f"sn{tagp}")
                nc.scalar.activation(out=sn[:], in_=z[:], func=AF.Sigmoid,
                                     scale=-1.0)
                nc.vector.reciprocal(out=sn[:], in_=sn[:])
                e = smallp.tile([128, n], FP32, tag=f"e{tagp}")
                nc.vector.tensor_tensor(out=e[:], in0=sp[:], in1=sn[:],
                                        op=ALU.mult)
                ssum = smallp.tile([128, 1], FP32, tag=f"ss{tagp}")
                nc.vector.tensor_reduce(out=ssum[:], in_=e[:],
                                        axis=mybir.AxisListType.X, op=ALU.add)
                nc.vector.reciprocal(out=ssum[:], in_=ssum[:])
                pn = smallp.tile([128, n], out_dtype, tag=f"pn{tagp}")
                nc.scalar.activation(out=pn[:], in_=e[:], func=AF.Copy,
                                     scale=ssum[:])
                return pn

            phn = softmax_relu(ps_ph, bh4, H, "h", FP32)
            pwn = softmax_relu(ps_pw, bw4, W, "w", BF16)

            # ---------------- amap + apply ----------------
            for k in range(32):  # 2 h-rows per psum chunk
                am = ps_amap_p.tile([128, 1024], FP32, tag="am")
                for hl in range(2):
                    h = 2 * k + hl
                    stat = statp.tile([128, 128], BF16, tag="stat")
                    nc.scalar.activation(out=stat[:], in_=bde[:], func=AF.Copy,
                                         scale=phn[:, h:h + 1])
                    nc.tensor.matmul(
                        am[:, hl * 512:(hl + 1) * 512], stat[:], pwn[:],
                        start=True, stop=True)
                nc.scalar.activation(out=am[:], in_=am[:], func=AF.Sigmoid,
                                     bias=be4, scale=1.0)
                if k % 2 == 0:
                    ost = outp.tile([128, 2048], FP32, tag="ost")
                hf, off = k // 16, (2 * k % 32) * 512
                q = k // 2
                if q % 4 == 3:
                    # gpsimd path: t1 = sigma+1 on ACT, multiply on GPSIMD
                    if k % 2 == 0:
                        t1 = t1p.tile([128, 2048], BF16, tag="t1")
                    nc.scalar.activation(
                        out=t1[:, (k % 2) * 1024:(k % 2 + 1) * 1024],
                        in_=am[:], func=AF.Copy, bias=1.0, scale=1.0)
                    if k % 2 == 1:
                        nc.gpsimd.tensor_tensor(
                            out=ost[:], in0=t1[:],
                            in1=xh[(s, hf)][:, off - 1024:off + 1024],
                            op=ALU.mult)
                else:
                    nc.vector.scalar_tensor_tensor(
                        out=ost[:, (k % 2) * 1024:(k % 2 + 1) * 1024],
                        in0=am[:], scalar=1.0,
                        in1=xh[(s, hf)][:, off:off + 1024],
                        op0=ALU.add, op1=ALU.mult)
                if k % 2 == 1:
                    nc.sync.dma_start(
                        out=yout2[s][:, q * 2048:(q + 1) * 2048], in_=ost[:])
    _dedupe_ldweights(nc)
    _split_sync_waits(nc)
    return nc


_CACHED = {}


def _get_program():
    if "nc" not in _CACHED:
        _install_ntff_hook()
        _CACHED["nc"] = _build_program()
    return _CACHED["nc"]


def _host_weights(w_h, b_h, w_w, b_w, w_e, b_e):
    # mean-pool 1/N scaling folded into the "avg" conv weight blocks
    bdh = np.empty((C, 6 * 128), np.float32)
    for dh in range(3):
        for k in range(2):
            m = w_h[:, :, dh, k].T.astype(np.float32)  # [c_in, o]
            if k == 0:
                m = m / np.float32(W)
            bdh[:, (dh * 2 + k) * 128:(dh * 2 + k + 1) * 128] = _blockdiag4(m)
    bdw = np.empty((C, 6 * 128), np.float32)
    for kh in range(2):
        for dw in range(3):
            m = w_w[:, :, kh, dw].T.astype(np.float32)
            if kh == 0:
                m = m / np.float32(H)
            jj = kh * 3 + dw
            bdw[:, jj * 128:(jj + 1) * 128] = _blockdiag4(m)
    bde = _blockdiag4(w_e[:, :, 0, 0].T.astype(np.float32))
    iden = np.eye(128, dtype=np.float32)
    bias = np.stack([np.tile(b_h, NPARTS), np.tile(b_w, NPARTS),
                     np.tile(b_e, NPARTS)], axis=1).astype(np.float32)
    return dict(bdh=bdh.astype(BF16NP), bdw=bdw.astype(BF16NP),
                bde=bde.astype(BF16NP), iden=iden.astype(BF16NP), bias=bias)


def _run(x, w_h, b_h, w_w, b_w, w_e, b_e, trace=False, tmpdir=None):
    nc = _get_program()
    wts = _host_weights(w_h, b_h, w_w, b_w, w_e, b_e)
    x = np.ascontiguousarray(x, dtype=np.float32)
    in_maps = []
    for i in range(NCORES):
        m = {"x": x[i * BPC:(i + 1) * BPC]}
        m.update(wts)
        in_maps.append(m)
    res = run_bass_kernel_spmd(nc, in_maps, list(range(NCORES)),
                               trace=trace, tmpdir=tmpdir)
    out = np.concatenate([res.results[i]["out"] for i in range(NCORES)],
                         axis=0)
    return out, res


def kernel(x, w_h, b_h, w_w, b_w, w_e, b_e):
    out, _ = _run(x, w_h, b_h, w_w, b_w, w_e, b_e, trace=False)
    return out


# revision 29
# speedup vs baseline: 1.0121x; 1.0089x over previous
"""Trainium2 Bass kernel for nn_EnhancedSpatialAttention.

Full (unsharded) inputs in, full output out. Internally: pure data-parallel
across 8 NeuronCores (2 batch samples per core), one Bass program run SPMD.

Per-sample layout on a core: x_s = [C=128 partitions, H=64, W=512].

x is cast fp32->bf16 during the input DMA (SWDGE cast); all matmuls run in
bf16 (fp32 matmuls on trn2 run in LOW_HIGH mode at ~3.4 cycles/column).
PSUM accumulation stays fp32, the softmax chain and the sigmoid/apply
stay fp32.

Per sample:
  pools:  h_max/w_max via DVE tensor_reduce; h_sum/w_sum via PE
          identity-matmul accumulation into PSUM (mean 1/N folded into the
          conv weights host-side)
  convs:  block-diagonal [128x128] bf16 stationaries (4 parts share
          weights), 6 accumulating matmuls each for ph / pw
  softmax(relu(z)): exp computed as sigmoid(z)/sigmoid(-z) so the whole
          kernel stays in the single "sigmoid" ACT table set
  amap:   per-h stationary (w_e^T * phn[:,h]) matmul against pwn
  apply:  out = x * (1 + sigmoid(amap + b_e)); sigmoid+1 on ACT,
          multiply on GPSIMD tensor_tensor (bf16 x, fp32 t, fp32 out)
"""

import os
import sys
import types
import contextlib

for _p in ("/opt/trn_rl_repo", "/root/.axon_site/_ro/trn_rl_repo"):
    if os.path.isdir(_p) and _p not in sys.path:
        sys.path.insert(0, _p)

import numpy as np
import ml_dtypes

import concourse.bass as bass
import concourse.tile as tile
from concourse import mybir
from concourse.tile import ScopedClock
import concourse.bass_utils as bass_utils
from concourse.bass_utils import run_bass_kernel_spmd

AF = mybir.ActivationFunctionType
ALU = mybir.AluOpType
FP32 = mybir.dt.float32
BF16 = mybir.dt.bfloat16
BF16NP = ml_dtypes.bfloat16

NCORES = 8
B, C, H, W = 16, 128, 64, 512
PC = 32          # channels per part
NPARTS = 4
BPC = B // NCORES  # samples per core
HW = H * W
HALF = HW // 2   # elements per half-sample (h rows 0..31 / 32..63)
HH = H // 2


def _patch_drain_split():
    """This container's walrus accepts only one sync-wait command per
    instruction; Tile's end-of-kernel drain carries one wait per live
    semaphore. Spread them across SP nops, one wait each."""
    if getattr(tile.TileContext, "_drain_split_patched", False):
        return

    def _drain_and_barrier_split(self, tick_clock, wait_clock):
        nc = self.nc
        probe = nc.sync.nop(hint="drain_wait_probe", nofuse=True)
        wait_clock.add_sem_waits(
            probe.ins, ScopedClock({None: tick_clock.global_clock})
        )
        waits = list(probe.ins.sync_info.on_wait or [])
        probe.ins.sync_info.on_wait = waits[:1]
        for w in waits[1:]:
            n = nc.sync.nop(hint="drain_wait_split", nofuse=True)
            n.ins.sync_info = mybir.SyncInfo(on_wait=[w], on_update=[])
        nc.sync.drain()
        nc.all_engine_barrier()
        assert self.sems is not None
        popped = nc._tile_sem_poison_stack.pop()
        assert popped is self._sem_poison
        nc.clear_and_free_semaphores(list(self.sems.allocated().values()))
        nc.all_engine_barrier()

    tile.TileContext._drain_and_barrier = _drain_and_barrier_split
    tile.TileContext._drain_split_patched = True


def _split_sync_waits(nc, max_waits=1):
    """This walrus build accepts at most one sync-wait command per
    instruction. Hoist extra waits onto same-engine NoOps inserted just
    before the instruction (the engine stalls on each in turn, which is
    semantically identical)."""
    for fn in nc.m.functions:
        for blk in fn.blocks:
            new = []
            for inst in blk.instructions:
                si = inst.sync_info
                if (si is not None and si.on_wait
                        and len(si.on_wait) > max_waits
                        and inst.engine != mybir.EngineType.Unassigned):
                    waits = list(si.on_wait)
                    for w in waits[max_waits:]:
                        nop = mybir.InstNoOp(
                            name=nc.get_next_instruction_name(),
                            engine=inst.engine,
                            ins=[], outs=[],
                            sync_info=mybir.SyncInfo(on_wait=[w], on_update=[]),
                        )
                        nc.register_instruction(nop, overwrite=True)
                        new.append(nop)
                    si.on_wait = waits[:max_waits]
                new.append(inst)
            blk.instructions[:] = new


def _dedupe_ldweights(nc):
    """Consecutive matmuls that reuse the same stationary (the identity for
    the pool-sum accumulations) do not need to reload it: LDWEIGHTS of the
    same rows serializes against the in-flight matmul, so each redundant
    reload costs a full array drain. Drop repeated LDWEIGHTS whose weights
    AP is identical to the previous one on the PE stream, carrying any
    sync waits onto the next kept PE instruction."""
    for fn in nc.m.functions:
        for blk in fn.blocks:
            new = []
            last_key = None
            pending = []
            for inst in blk.instructions:
                if inst.engine == mybir.EngineType.PE:
                    tn = type(inst).__name__
                    if tn == "InstLdweights":
                        a = inst.ins[0]
                        key = (getattr(a, "memref", None), a.offset,
                               str(a.ap), str(a.dtype))
                        has_upd = bool(inst.sync_info
                                       and inst.sync_info.on_update)
                        if key == last_key and not has_upd:
                            if inst.sync_info and inst.sync_info.on_wait:
                                pending.extend(inst.sync_info.on_wait)
                            continue
                        last_key = key
                    elif tn != "InstMatmult":
                        last_key = None
                    if pending:
                        si = inst.sync_info
                        if si is None:
                            inst.sync_info = mybir.SyncInfo(
                                on_wait=list(pending), on_update=[])
                        else:
                            si.on_wait = list(si.on_wait) + pending
                        pending = []
                new.append(inst)
            blk.instructions[:] = new


def _install_ntff_hook():
    """run_bass_kernel_spmd(trace=True) imports antenv.axon_hooks, which is
    absent in this container; provide it, backed by the ctypes NTFF hook
    from trn_agent_boot. Harmless if tracing is never requested."""
    if "antenv.axon_hooks" in sys.modules:
        return
    mod = types.ModuleType("antenv.axon_hooks")
    holder = [None]
    mod.set_axon_ntff_profile_hook = lambda h: holder.__setitem__(0, h)
    mod.get_axon_ntff_profile_hook = lambda: holder[0]
    sys.modules["antenv.axon_hooks"] = mod
    try:
        from trn_agent_boot.trn_boot import _ntff_profile_via_ctypes

        so = "/opt/axon/libaxon_pjrt.so"
        if os.path.exists(so):
            holder[0] = _ntff_profile_via_ctypes(so)
    except Exception:
        pass
    # upload_artifacts needs S3; keep artifacts local.
    bass_utils.upload_artifacts = lambda tmpdir: "file://" + tmpdir


def _blockdiag4(m32):
    out = np.zeros((128, 128), np.float32)
    for p in range(NPARTS):
        out[p * PC:(p + 1) * PC, p * PC:(p + 1) * PC] = m32
    return out


def _build_program():
    _patch_drain_split()
    nc = bass.Bass()
    xin = nc.declare_dram_parameter("x", [BPC, C, H, W], FP32, isOutput=False)
    bdh_d = nc.declare_dram_parameter("bdh", [C, 6 * 128], BF16, isOutput=False)
    bdw_d = nc.declare_dram_parameter("bdw", [C, 6 * 128], BF16, isOutput=False)
    bde_d = nc.declare_dram_parameter("bde", [C, 128], BF16, isOutput=False)
    iden_d = nc.declare_dram_parameter("iden", [C, 128], BF16, isOutput=False)
    bias_d = nc.declare_dram_parameter("bias", [C, 3], FP32, isOutput=False)
    yout = nc.declare_dram_parameter("out", [BPC, C, H, W], FP32, isOutput=True)

    with tile.TileContext(nc) as tc, contextlib.ExitStack() as ctx:
        consts = ctx.enter_context(tc.tile_pool(name="consts", bufs=1))
        xpool = ctx.enter_context(tc.tile_pool(name="xh", bufs=3))
        t1p = ctx.enter_context(tc.tile_pool(name="t1", bufs=2))
        foldp = ctx.enter_context(tc.tile_pool(name="fold", bufs=1))
        outp = ctx.enter_context(tc.tile_pool(name="ost", bufs=3))
        statp = ctx.enter_context(tc.tile_pool(name="stat", bufs=4))
        smallp = ctx.enter_context(tc.tile_pool(name="small", bufs=1))
        ps_wsum_p = ctx.enter_context(
            tc.tile_pool(name="ps_wsum", bufs=1, space="PSUM"))
        ps_conv_p = ctx.enter_context(
            tc.tile_pool(name="ps_conv", bufs=1, space="PSUM"))
        ps_amap_p = ctx.enter_context(
            tc.tile_pool(name="ps_amap", bufs=2, space="PSUM"))

        # constants
        bdh = consts.tile([128, 6 * 128], BF16)
        nc.gpsimd.dma_start(out=bdh[:], in_=bdh_d[:])
        bdw = consts.tile([128, 6 * 128], BF16)
        nc.gpsimd.dma_start(out=bdw[:], in_=bdw_d[:])
        bde = consts.tile([128, 128], BF16)
        nc.gpsimd.dma_start(out=bde[:], in_=bde_d[:])
        iden = consts.tile([128, 128], BF16)
        nc.gpsimd.dma_start(out=iden[:], in_=iden_d[:])
        biases = consts.tile([128, 3], FP32)
        nc.gpsimd.dma_start(out=biases[:], in_=bias_d[:])
        bh4 = biases[:, 0:1]
        bw4 = biases[:, 1:2]
        be4 = biases[:, 2:3]

        xin2 = [xin[s].rearrange("c h w -> c (h w)") for s in range(BPC)]
        yout2 = [yout[s].rearrange("c h w -> c (h w)") for s in range(BPC)]

        # All input DMAs issued upfront (SWDGE drains its queue FIFO, so
        # sample 0's halves complete first; SDMA transfer is async).
        # fp32 -> bf16 cast happens inline in the DMA.
        xh = {}
        QRT = HALF // 8
        for s in range(BPC):
            for hf in range(2):
                xt = xpool.tile([128, HALF], BF16, tag="xh")
                for q in range(8):
                    nc.gpsimd.dma_start(
                        out=xt[:, q * QRT:(q + 1) * QRT],
                        in_=xin2[s][:, hf * HALF + q * QRT:
                                    hf * HALF + (q + 1) * QRT])
                xh[(s, hf)] = xt

        for s in range(BPC):
            # ---------------- reduce phase ----------------
            # pool buffers (zero-padded boundary columns)
            ph_sum = smallp.tile([128, H + 2], BF16, tag="ph_sum")
            ph_max = smallp.tile([128, H + 2], BF16, tag="ph_max")
            pw_sum = smallp.tile([128, W + 2], BF16, tag="pw_sum")
            pw_max = smallp.tile([128, W + 2], BF16, tag="pw_max")
            for t, n in ((ph_sum, H), (ph_max, H), (pw_sum, W), (pw_max, W)):
                nc.vector.memset(t[:, 0:1], 0.0)
                nc.vector.memset(t[:, n + 1:n + 2], 0.0)

            ps_wsum = ps_wsum_p.tile([128, W], FP32, tag="ps_wsum")
            wmax_part = []
            for hf in range(2):
                xt = xh[(s, hf)]
                x3 = xt.rearrange("p (h w) -> p h w", h=HH)
                # h_max rows for this half: fold w 512->64, then reduce
                ha = foldp.tile([128, HALF // 2], BF16, tag="fold_a")
                h3a = ha.rearrange("p (h w) -> p h w", h=HH)
                nc.vector.tensor_tensor(
                    out=h3a[:], in0=x3[:, :, 0:256], in1=x3[:, :, 256:512],
                    op=ALU.max)
                hb = foldp.tile([128, HALF // 4], BF16, tag="fold_b")
                h3b = hb.rearrange("p (h w) -> p h w", h=HH)
                nc.vector.tensor_tensor(
                    out=h3b[:], in0=h3a[:, :, 0:128], in1=h3a[:, :, 128:256],
                    op=ALU.max)
                hc = foldp.tile([128, HALF // 8], BF16, tag="fold_c")
                h3c = hc.rearrange("p (h w) -> p h w", h=HH)
                nc.vector.tensor_tensor(
                    out=h3c[:], in0=h3b[:, :, 0:64], in1=h3b[:, :, 64:128],
                    op=ALU.max)
                nc.vector.tensor_reduce(
                    out=ph_max[:, 1 + hf * HH:1 + (hf + 1) * HH],
                    in_=h3c, axis=mybir.AxisListType.X, op=ALU.max)
                # w_max partial via pairwise fold tree over h (contiguous)
                fa = foldp.tile([128, HALF // 2], BF16, tag="fold_a")
                nc.vector.tensor_tensor(
                    out=fa[:], in0=xt[:, :HALF // 2], in1=xt[:, HALF // 2:],
                    op=ALU.max)
                fb = foldp.tile([128, HALF // 4], BF16, tag="fold_b")
                nc.vector.tensor_tensor(
                    out=fb[:], in0=fa[:, :HALF // 4], in1=fa[:, HALF // 4:],
                    op=ALU.max)
                fc = foldp.tile([128, HALF // 8], BF16, tag="fold_c")
                nc.vector.tensor_tensor(
                    out=fc[:], in0=fb[:, :HALF // 8], in1=fb[:, HALF // 8:],
                    op=ALU.max)
                fd = foldp.tile([128, HALF // 16], BF16, tag="fold_d")
                nc.vector.tensor_tensor(
                    out=fd[:], in0=fc[:, :HALF // 16], in1=fc[:, HALF // 16:],
                    op=ALU.max)
                pm = smallp.tile([128, W], BF16, tag=f"wmax{hf}")
                nc.vector.tensor_tensor(
                    out=pm[:], in0=fd[:, :W], in1=fd[:, W:], op=ALU.max)
                wmax_part.append(pm)
                # w_sum: accumulate h-row slices
                for h in range(HH):
                    nc.tensor.matmul(
                        ps_wsum[:], iden[:], x3[:, h, :],
                        start=(hf == 0 and h == 0),
                        stop=(hf == 1 and h == HH - 1))
                # h_sum rows: bf16 add-fold w 512->64, then one fp32 reduce
                sa = foldp.tile([128, HALF // 2], BF16, tag="sfold_a")
                s3a = sa.rearrange("p (h w) -> p h w", h=HH)
                nc.vector.tensor_tensor(
                    out=s3a[:], in0=x3[:, :, 0:256], in1=x3[:, :, 256:512],
                    op=ALU.add)
                sb = foldp.tile([128, HALF // 4], BF16, tag="sfold_b")
                s3b = sb.rearrange("p (h w) -> p h w", h=HH)
                nc.vector.tensor_tensor(
                    out=s3b[:], in0=s3a[:, :, 0:128], in1=s3a[:, :, 128:256],
                    op=ALU.add)
                sc = foldp.tile([128, HALF // 8], BF16, tag="sfold_c")
                s3c = sc.rearrange("p (h w) -> p h w", h=HH)
                nc.vector.tensor_tensor(
                    out=s3c[:], in0=s3b[:, :, 0:64], in1=s3b[:, :, 64:128],
                    op=ALU.add)
                hs = smallp.tile([128, HH], FP32, tag=f"hs{hf}")
                nc.vector.tensor_reduce(
                    out=hs[:], in_=s3c, axis=mybir.AxisListType.X, op=ALU.add)
                nc.scalar.activation(
                    out=ph_sum[:, 1 + hf * HH:1 + (hf + 1) * HH],
                    in_=hs[:], func=AF.Copy)
            # w_max combine + w_sum pair-combine
            nc.vector.tensor_tensor(
                out=pw_max[:, 1:1 + W], in0=wmax_part[0][:],
                in1=wmax_part[1][:], op=ALU.max)
            nc.scalar.activation(
                out=pw_sum[:, 1:1 + W], in_=ps_wsum[:], func=AF.Copy)

            # ---------------- convs + softmax ----------------
            ps_ph = ps_conv_p.tile([128, H], FP32, tag="ps_ph")
            ps_pw = ps_conv_p.tile([128, W], FP32, tag="ps_pw")
            for j, (dh, k) in enumerate(
                    (dh, k) for dh in range(3) for k in range(2)):
                pool = ph_sum if k == 0 else ph_max
                nc.tensor.matmul(
                    ps_ph[:], bdh[:, j * 128:(j + 1) * 128],
                    pool[:, dh:dh + H], start=(j == 0), stop=(j == 5))
            for kh in range(2):
                for dw in range(3):
                    pool = pw_sum if kh == 0 else pw_max
                    jj = kh * 3 + dw
                    nc.tensor.matmul(
                        ps_pw[:], bdw[:, jj * 128:(jj + 1) * 128],
                        pool[:, dw:dw + W], start=(jj == 0), stop=(jj == 5))

            def softmax_relu(ps, bias_ap, n, tagp, out_dtype):
                z = smallp.tile([128, n], FP32, tag=f"z{tagp}")
                nc.scalar.activation(out=z[:], in_=ps[:], func=AF.Relu,
                                     bias=bias_ap, scale=1.0)
                sp = smallp.tile([128, n], FP32, tag=f"sp{tagp}")
                nc.scalar.activation(out=sp[:], in_=z[:], func=AF.Sigmoid)
                sn = smallp.tile([128, n], FP32, tag=f"sn{tagp}")
                nc.scalar.activation(out=sn[:], in_=z[:], func=AF.Sigmoid,
                                     scale=-1.0)
                nc.vector.reciprocal(out=sn[:], in_=sn[:])
                e = smallp.tile([128, n], FP32, tag=f"e{tagp}")
                nc.vector.tensor_tensor(out=e[:], in0=sp[:], in1=sn[:],
                                        op=ALU.mult)
                ssum = smallp.tile([128, 1], FP32, tag=f"ss{tagp}")
                nc.vector.tensor_reduce(out=ssum[:], in_=e[:],
                                        axis=mybir.AxisListType.X, op=ALU.add)
                nc.vector.reciprocal(out=ssum[:], in_=ssum[:])
                pn = smallp.tile([128, n], out_dtype, tag=f"pn{tagp}")
                nc.scalar.activation(out=pn[:], in_=e[:], func=AF.Copy,
                                     scale=ssum[:])
                return pn

            phn = softmax_relu(ps_ph, bh4, H, "h", FP32)
            pwn = softmax_relu(ps_pw, bw4, W, "w", BF16)

            # ---------------- amap + apply ----------------
            for k in range(32):  # 2 h-rows per psum chunk
                am = ps_amap_p.tile([128, 1024], FP32, tag="am")
                for hl in range(2):
                    h = 2 * k + hl
                    stat = statp.tile([128, 128], BF16, tag="stat")
                    nc.scalar.activation(out=stat[:], in_=bde[:], func=AF.Copy,
                                         scale=phn[:, h:h + 1])
                    nc.tensor.matmul(
                        am[:, hl * 512:(hl + 1) * 512], stat[:], pwn[:],
                        start=True, stop=True)
                nc.scalar.activation(out=am[:], in_=am[:], func=AF.Sigmoid,
                                     bias=be4, scale=1.0)
                if k % 2 == 0:
                    ost = outp.tile([128, 2048], FP32, tag="ost")
                hf, off = k // 16, (2 * k % 32) * 512
                q = k // 2
                if q % 4 == 3:
                    # gpsimd path: t1 = sigma+1 on ACT, multiply on GPSIMD
                    if k % 2 == 0:
                        t1 = t1p.tile([128, 2048], BF16, tag="t1")
                    nc.scalar.activation(
                        out=t1[:, (k % 2) * 1024:(k % 2 + 1) * 1024],
                        in_=am[:], func=AF.Copy, bias=1.0, scale=1.0)
                    if k % 2 == 1:
                        nc.gpsimd.tensor_tensor(
                            out=ost[:], in0=t1[:],
                            in1=xh[(s, hf)][:, off - 1024:off + 1024],
                            op=ALU.mult)
                else:
                    nc.vector.scalar_tensor_tensor(
                        out=ost[:, (k % 2) * 1024:(k % 2 + 1) * 1024],
                        in0=am[:], scalar=1.0,
                        in1=xh[(s, hf)][:, off:off + 1024],
                        op0=ALU.add, op1=ALU.mult)
                if k % 2 == 1:
                    nc.sync.dma_start(
                        out=yout2[s][:, q * 2048:(q + 1) * 2048], in_=ost[:])
    _dedupe_ldweights(nc)
    _split_sync_waits(nc)
    return nc


_CACHED = {}


def _get_program():
    if "nc" not in _CACHED:
        _install_ntff_hook()
        _CACHED["nc"] = _build_program()
    return _CACHED["nc"]


def _host_weights(w_h, b_h, w_w, b_w, w_e, b_e):
    # mean-pool 1/N scaling folded into the "avg" conv weight blocks
    bdh = np.empty((C, 6 * 128), np.float32)
    for dh in range(3):
        for k in range(2):
            m = w_h[:, :, dh, k].T.astype(np.float32)  # [c_in, o]
            if k == 0:
                m = m / np.float32(W)
            bdh[:, (dh * 2 + k) * 128:(dh * 2 + k + 1) * 128] = _blockdiag4(m)
    bdw = np.empty((C, 6 * 128), np.float32)
    for kh in range(2):
        for dw in range(3):
            m = w_w[:, :, kh, dw].T.astype(np.float32)
            if kh == 0:
                m = m / np.float32(H)
            jj = kh * 3 + dw
            bdw[:, jj * 128:(jj + 1) * 128] = _blockdiag4(m)
    bde = _blockdiag4(w_e[:, :, 0, 0].T.astype(np.float32))
    iden = np.eye(128, dtype=np.float32)
    bias = np.stack([np.tile(b_h, NPARTS), np.tile(b_w, NPARTS),
                     np.tile(b_e, NPARTS)], axis=1).astype(np.float32)
    return dict(bdh=bdh.astype(BF16NP), bdw=bdw.astype(BF16NP),
                bde=bde.astype(BF16NP), iden=iden.astype(BF16NP), bias=bias)


def _run(x, w_h, b_h, w_w, b_w, w_e, b_e, trace=False, tmpdir=None):
    nc = _get_program()
    wts = _host_weights(w_h, b_h, w_w, b_w, w_e, b_e)
    x = np.ascontiguousarray(x, dtype=np.float32)
    in_maps = []
    for i in range(NCORES):
        m = {"x": x[i * BPC:(i + 1) * BPC]}
        m.update(wts)
        in_maps.append(m)
    res = run_bass_kernel_spmd(nc, in_maps, list(range(NCORES)),
                               trace=trace, tmpdir=tmpdir)
    out = np.concatenate([res.results[i]["out"] for i in range(NCORES)],
                         axis=0)
    return out, res


def kernel(x, w_h, b_h, w_w, b_w, w_e, b_e):
    out, _ = _run(x, w_h, b_h, w_w, b_w, w_e, b_e, trace=False)
    return out
